# revision 4
# baseline (speedup 1.0000x reference)
"""ConvDeepSet Trainium2 kernel, v3 (streaming RBF, minimal-DMA).

Same math as v2 (RBF grid factorization, fp16 single precision, host
m-permutation, fp16 device output) but engineered around the ~600ns
per-DMA trigger cost and the shared HWDGE:

  - 14 DMA instructions total: 4 inputs, 4 feats-compaction per batch
    (x2), 1 output per batch (x2), + few misc.
  - A [128, 2] replicated in PSUM by 16 tiny matmuls (no DMAs).
  - Per 2048-m block: 4 D2 MMs -> exp [128,512]->fp16 -> 4 agg MMs ->
    one psum->sbuf f32 copy into a per-batch staging tile.
  - Per batch: 4 compaction DMAs build feats [64+4, 128] f32
    (row = 16s + 8ch + 4B + j; rows 64-67 = ones), stream_shuffle
    duplicates dens under conv rows (ones under dens rows), one
    fp32 fast reciprocal, one full multiply -> feats fp16 [64, 128].
  - Finale: 4 MMs (lhsT = feats [64,128], rhs = block-diagonal
    W [64, 2048] fp16) -> 2x [128,1024] psum -> fp16 osb [128, 2048]
    -> ONE output DMA (2KB contiguous runs per partition).

m map: m_global = 2048*B + 16*p + 4*j + s   (B block, j colblock,
s stream, p partition); feats row for chunk (B, j, s):
dens = 16s + 4*B + j, norm = 16s + 8 + 4*B + j.
"""

import numpy as np
import ml_dtypes

import concourse.bass as bass
import concourse.bacc as bacc
import concourse.tile as tile
import concourse.mybir as mybir
from concourse.bass_utils import run_bass_kernel_spmd

B, N_IN, N_OUT = 16, 512, 4096
OUT_CH = 64
N_CORES = 8
BPC = B // N_CORES
P = 128
GRID = 32
NXCH = N_IN // P  # 4
NBLK = 2  # blocks per batch
BLK = N_OUT // NBLK  # 2048
F32 = mybir.dt.float32
BF16 = mybir.dt.bfloat16
FP16 = mybir.dt.float16
BF = ml_dtypes.bfloat16

_CACHE: dict = {}

# stream_shuffle mask: 32 per-quadrant lane entries, same permutation in
# every 32-partition quadrant; 255 = suppress write (dest keeps old value).
# feats row-in-quadrant r = 16*sigma + 8*ch + 4*B + j: conv lanes
# (r%16 >= 8) pull dens from r-8; all other lanes keep the memset 1.0.
_SHUF_MASK = [(_i - 8) if (_i % 16) >= 8 else 255 for _i in range(32)]


def _build_rbf3(ln_c0: float):
    nc = bacc.Bacc("TRN2", target_bir_lowering=False, debug=False)

    dlr_d = nc.dram_tensor(
        "dlr", [BPC, 12, N_IN + N_OUT], BF16, kind="ExternalInput"
    ).ap()
    dgrid_d = nc.dram_tensor("dgrid", [12, 2 * GRID], BF16, kind="ExternalInput").ap()
    y2_d = nc.dram_tensor("y2", [BPC, P, NXCH, 2], FP16, kind="ExternalInput").ap()
    wbig_d = nc.dram_tensor("wbig", [64, 2048], FP16, kind="ExternalInput").ap()
    out_d = nc.dram_tensor("out", [BPC, N_OUT, OUT_CH], FP16, kind="ExternalOutput").ap()

    with tile.TileContext(nc) as tc:
        with (
            tc.tile_pool(name="singles", bufs=1) as singles,
            tc.tile_pool(name="phip", bufs=4) as phip,
            tc.tile_pool(name="fcq", bufs=2) as fcq,
            tc.tile_pool(name="featp", bufs=2) as featp,
            tc.tile_pool(name="osbp", bufs=2) as osbp,
            tc.tile_pool(name="dps", bufs=2, space="PSUM") as dps,
            tc.tile_pool(name="aggp", bufs=2, space="PSUM") as aggp,
            tc.tile_pool(name="fop", bufs=2, space="PSUM") as fop,
        ):
            lnc0_sb = singles.tile([P, 1], F32)
            nc.vector.memset(lnc0_sb, ln_c0)
            dummy = singles.tile([1, 1], F32)
            nc.vector.memset(dummy, 0.0)
            dummy2 = singles.tile([1, 1], F32)
            nc.scalar.activation(
                out=dummy2, in_=dummy, func=mybir.ActivationFunctionType.Exp
            )
            nc.vector.tensor_copy(dummy, dummy2)  # keep dummy2 read

            # input DMAs spread across engines, dlr (the gating one) first
            dlr_all = singles.tile([12, BPC * (N_IN + N_OUT)], BF16)
            NLR = N_IN + N_OUT
            for bb in range(BPC):
                eng = nc.sync if bb == 0 else nc.scalar
                eng.dma_start(
                    out=dlr_all[:, bb * NLR : (bb + 1) * NLR],
                    in_=dlr_d[bb],
                )
            dgrid_sb = singles.tile([12, 2 * GRID], BF16)
            nc.sync.dma_start(out=dgrid_sb, in_=dgrid_d, single_packet=True)
            wbig_sb = singles.tile([64, 2048], FP16)
            nc.gpsimd.dma_start(out=wbig_sb, in_=wbig_d)
            d1r_sb = dgrid_sb[:, 0:GRID]
            d2l_sb = dgrid_sb[:, GRID : 2 * GRID]
            y2_all = singles.tile([P, BPC, NXCH, 2], FP16)
            nc.gpsimd.dma_start(
                out=y2_all,
                in_=bass.AP(
                    tensor=y2_d.tensor,
                    offset=y2_d.offset,
                    ap=[y2_d.ap[1], y2_d.ap[0], y2_d.ap[2], y2_d.ap[3]],
                ),
            )

            af16 = []
            for bb in range(BPC):
                a = singles.tile([P, 2], FP16, name=f"af16_{bb}")
                af16.append(a)

            def prologue(bb):
                base = bb * (N_IN + N_OUT)
                d1l_sb = dlr_all[:, base : base + N_IN]
                d1ps = fop.tile([P, 1024], F32, tag="fo", name="d1ps")
                for k in range(NXCH):
                    nc.tensor.matmul(
                        d1ps[:, 32 * k : 32 * k + GRID],
                        d1l_sb[:, k * P : (k + 1) * P],
                        d1r_sb,
                        start=True,
                        stop=True,
                    )
                phx = phip.tile([P, P], FP16, tag="phx")
                nc.scalar.activation(
                    out=phx,
                    in_=d1ps[:, 0:P],
                    func=mybir.ActivationFunctionType.Exp,
                    scale=-1.0,
                    bias=lnc0_sb,
                )
                # A accumulated 4x replicated along partitions: 16 tiny MMs
                aps = aggp.tile([P, 512], F32, tag="agg", name="aps")
                for r in range(4):
                    for k in range(NXCH):
                        nc.tensor.matmul(
                            aps[32 * r : 32 * r + GRID, 0:2],
                            phx[:, 32 * k : 32 * k + GRID],
                            y2_all[:, bb, k, :],
                            start=(k == 0),
                            stop=(k == NXCH - 1),
                            tile_position=(0, 32 * r),
                        )
                nc.scalar.copy(af16[bb], aps[:, 0:2])

            def block(bb, blk, fcomp):
                base = bb * (N_IN + N_OUT) + N_IN + blk * BLK
                d2ps = dps.tile([P, 512], F32, tag="d2")
                for s in range(4):
                    nc.tensor.matmul(
                        d2ps[32 * s : 32 * s + GRID, :],
                        d2l_sb,
                        dlr_all[:, base + 512 * s : base + 512 * (s + 1)],
                        start=True,
                        stop=True,
                        tile_position=(0, 32 * s),
                    )
                phi = phip.tile([P, 512], FP16, tag="phi")
                nc.scalar.activation(
                    out=phi,
                    in_=d2ps,
                    func=mybir.ActivationFunctionType.Exp,
                    scale=-1.0,
                )
                aggps = aggp.tile([P, 512], F32, tag="agg")
                for s in range(4):
                    nc.tensor.matmul(
                        aggps[32 * s : 32 * s + 2, :],
                        af16[bb][32 * s : 32 * s + GRID, :],
                        phi[32 * s : 32 * s + GRID, :],
                        start=True,
                        stop=True,
                        tile_position=(32 * s, 32 * s),
                    )
                dst = fcomp[:, 512 * blk : 512 * (blk + 1)]
                if blk % 2 == 0:
                    nc.scalar.copy(dst, aggps)
                else:
                    nc.vector.tensor_copy(dst, aggps)
            def batch_tail(bb, fcomp):
                # compaction: per s, one DMA moving both blocks+channels:
                # feats row 16s+8ch+4B+j <- fcomp[32s+ch, 512B+128j+p]
                feats = feats_of[bb]
                pf = fcomp.ap[0][0]
                pt = feats.ap[0][0]
                for s in range(4):
                    src = bass.AP(
                        tensor=fcomp.tensor,
                        offset=fcomp.offset + 32 * s * pf,
                        ap=[[pf, 2], [512, 2], [128, 4], [1, P]],
                    )
                    dstf = bass.AP(
                        tensor=feats.tensor,
                        offset=feats.offset + 16 * s * pt,
                        ap=[[pt, 16], [1, P]],
                    )
                    eng = nc.sync if s % 2 == 0 else nc.scalar
                    eng.dma_start(out=dstf, in_=src)
                # normalize: ddup = 1.0 except conv lanes get dens (row-8);
                # rall = 1/ddup; feats * rall -> fp16 (dens*1, conv/dens)
                ddup = featp.tile([64, P], F32, tag="dd")
                nc.gpsimd.memset(ddup, 1.0)
                nc.vector.stream_shuffle(ddup, feats, _SHUF_MASK)
                rall = featp.tile([64, P], F32, tag="ra")
                nc.vector.reciprocal_approx_fast(out=rall, in_=ddup)
                featsf = featp.tile([64, P], FP16, tag="ff")
                nc.vector.tensor_mul(featsf, feats, rall)
                # finale: 4 MMs vs block-diagonal W, 2x [128,1024] psum
                osb = osbp.tile([P, 2048], FP16, tag="osb")
                for h in range(2):
                    fout = fop.tile([P, 1024], F32, tag="fo")
                    for u in range(2):
                        for g in range(2):
                            nc.tensor.matmul(
                                fout[64 * g : 64 * (g + 1), 512 * u : 512 * (u + 1)],
                                featsf[:, 64 * g : 64 * (g + 1)],
                                wbig_sb[
                                    :, 1024 * h + 512 * u : 1024 * h + 512 * (u + 1)
                                ],
                                start=True,
                                stop=True,
                                tile_position=(0, 64 * g),
                            )
                    for u in range(2):
                        dsl = osb[:, 1024 * h + 512 * u : 1024 * h + 512 * (u + 1)]
                        if (h + u) % 2 == 0:
                            nc.scalar.copy(dsl, fout[:, 512 * u : 512 * (u + 1)])
                        else:
                            nc.vector.tensor_copy(
                                dsl, fout[:, 512 * u : 512 * (u + 1)]
                            )
                    # output DMA per half (one 2048-m block each)
                    sub = out_d[bb]
                    dsto = bass.AP(
                        tensor=sub.tensor,
                        offset=sub.offset + h * BLK * OUT_CH,
                        ap=[[16 * OUT_CH, P], [1, 1024]],
                    )
                    eng = nc.sync if h == 0 else nc.gpsimd
                    eng.dma_start(out=dsto, in_=osb[:, 1024 * h : 1024 * (h + 1)])

            feats_of = [featp.tile([64, P], F32, tag="ft", name=f"feats{i}") for i in range(BPC)]
            for bb in range(BPC):
                prologue(bb)
            for bb in range(BPC):
                fcomp = fcq.tile([P, 1024], F32, tag="fq", name=f"fcomp{bb}")
                for blk in range(NBLK):
                    block(bb, blk, fcomp)
                batch_tail(bb, fcomp)

    nc.compile()
    return nc


def _split3(v):
    vh = v.astype(BF)
    r1 = v - vh.astype(np.float64)
    vm = r1.astype(BF)
    r2 = r1 - vm.astype(np.float64)
    vl = r2.astype(BF)
    return vh, vm, vl


def _d_rows(a, pts_t, pts_x):
    t = np.asarray(pts_t, dtype=np.float64)
    x = np.asarray(pts_x, dtype=np.float64)
    t2h, t2m, t2l = _split3(a * t * t)
    x2h, x2m, x2l = _split3(a * x * x)
    th, tm, tl = _split3(t)
    uh, um, ul = _split3(-2.0 * a * x)
    ones_t = np.ones_like(t, dtype=BF)
    ones_x = np.ones_like(x, dtype=BF)
    lhs = np.stack(
        [t2h, t2m, t2l, ones_t, ones_t, ones_t, th, th, tm, th, tm, tl], axis=-2
    )
    rhs = np.stack(
        [ones_x, ones_x, ones_x, x2h, x2m, x2l, uh, um, uh, ul, um, uh], axis=-2
    )
    return np.ascontiguousarray(lhs), np.ascontiguousarray(rhs)


def _prep_rbf3(x, y, t, a0, W):
    beta = 2.0 * a0
    s = 1.0 / (2.0 * np.sqrt(a0))
    margin = s * 5.68
    g = np.linspace(-margin, 1.0 + margin, GRID)
    h = g[1] - g[0]
    c0 = h * np.sqrt(4.0 * a0 / np.pi)
    ln_c0 = float(np.log(c0))

    # col (B*4+s)*512 + 128*j + p  <-  m = 2048*B + 16*p + 4*j + s
    tp = t.reshape(B, NBLK, P, 4, 4).transpose(0, 1, 4, 3, 2).reshape(B, N_OUT)
    d1_lhs, d1_rhs = _d_rows(beta, x, g)
    d2_lhs, d2_rhs = _d_rows(beta, g, tp)
    dlr = np.ascontiguousarray(np.concatenate([d1_lhs, d2_rhs], axis=-1))
    dgrid = np.ascontiguousarray(np.concatenate([d1_rhs, d2_lhs], axis=-1))
    y2 = np.empty((B, P, NXCH, 2), np.float16)
    y2[..., 0] = 1.0
    y2[..., 1] = y.reshape(B, NXCH, P).transpose(0, 2, 1)
    # wbig [64, 2048]: col block 64*(16B+4j+s): dens row 16s+4B+j = W0,
    # norm row 16s+8+4B+j = W1
    wbig = np.zeros((64, 2048), np.float16)
    w0 = W[:, 0].astype(np.float16)
    w1 = W[:, 1].astype(np.float16)
    for Bq in range(2):
        for j in range(4):
            for sq in range(4):
                cb = 64 * (16 * Bq + 4 * j + sq)
                wbig[16 * sq + 4 * Bq + j, cb : cb + 64] = w0
                wbig[16 * sq + 8 + 4 * Bq + j, cb : cb + 64] = w1

    in_maps = []
    for c in range(N_CORES):
        sl = slice(c * BPC, (c + 1) * BPC)
        in_maps.append(
            {
                "dlr": dlr[sl],
                "dgrid": dgrid,
                "y2": np.ascontiguousarray(y2[sl]),
                "wbig": wbig,
            }
        )
    return in_maps, ln_c0


def kernel(x, y, t, sigma, W, b, _trace=False):
    x = np.ascontiguousarray(x[..., 0], dtype=np.float32)
    y = np.ascontiguousarray(y[..., 0], dtype=np.float32)
    t = np.ascontiguousarray(t[..., 0], dtype=np.float32)
    scales = np.exp(sigma.astype(np.float32))
    a0 = float(np.float32(0.5) / (scales[0] * scales[0]))
    a1 = float(np.float32(0.5) / (scales[1] * scales[1]))
    assert a0 == a1, "v3 kernel requires shared length scale"

    in_maps, ln_c0 = _prep_rbf3(x, y, t, a0, W)
    key = ("rbf3", ln_c0)
    if key not in _CACHE:
        _CACHE[key] = _build_rbf3(ln_c0)
    nc = _CACHE[key]
    res = run_bass_kernel_spmd(
        nc, in_maps, core_ids=list(range(N_CORES)), trace=_trace
    )
    out = np.concatenate([r["out"] for r in res.results], axis=0)
    kernel.last_exec_time_ns = res.exec_time_ns
    kernel.last_results = res
    out = out.reshape(B, N_OUT, OUT_CH).astype(np.float32)
    out += b.astype(np.float32)[None, None, :]
    return np.ascontiguousarray(out)


# revision 5
# speedup vs baseline: 1.0169x; 1.0169x over previous
"""ConvDeepSet Trainium2 kernel, v3 (streaming RBF, minimal-DMA).

Same math as v2 (RBF grid factorization, fp16 single precision, host
m-permutation, fp16 device output) but engineered around the ~600ns
per-DMA trigger cost and the shared HWDGE:

  - 14 DMA instructions total: 4 inputs, 4 feats-compaction per batch
    (x2), 1 output per batch (x2), + few misc.
  - A [128, 2] replicated in PSUM by 16 tiny matmuls (no DMAs).
  - Per 2048-m block: 4 D2 MMs -> exp [128,512]->fp16 -> 4 agg MMs ->
    one psum->sbuf f32 copy into a per-batch staging tile.
  - Per batch: 4 compaction DMAs build feats [64+4, 128] f32
    (row = 16s + 8ch + 4B + j; rows 64-67 = ones), stream_shuffle
    duplicates dens under conv rows (ones under dens rows), one
    fp32 fast reciprocal, one full multiply -> feats fp16 [64, 128].
  - Finale: 4 MMs (lhsT = feats [64,128], rhs = block-diagonal
    W [64, 2048] fp16) -> 2x [128,1024] psum -> fp16 osb [128, 2048]
    -> ONE output DMA (2KB contiguous runs per partition).

m map: m_global = 2048*B + 16*p + 4*j + s   (B block, j colblock,
s stream, p partition); feats row for chunk (B, j, s):
dens = 16s + 4*B + j, norm = 16s + 8 + 4*B + j.
"""

import numpy as np
import ml_dtypes

import concourse.bass as bass
import concourse.bacc as bacc
import concourse.tile as tile
import concourse.mybir as mybir
from concourse.bass_utils import run_bass_kernel_spmd

B, N_IN, N_OUT = 16, 512, 4096
OUT_CH = 64
N_CORES = 8
BPC = B // N_CORES
P = 128
GRID = 32
NXCH = N_IN // P  # 4
NBLK = 2  # blocks per batch
BLK = N_OUT // NBLK  # 2048
F32 = mybir.dt.float32
BF16 = mybir.dt.bfloat16
FP16 = mybir.dt.float16
BF = ml_dtypes.bfloat16

_CACHE: dict = {}

# stream_shuffle mask: 32 per-quadrant lane entries, same permutation in
# every 32-partition quadrant; 255 = suppress write (dest keeps old value).
# feats row-in-quadrant r = 16*sigma + 8*ch + 4*B + j: conv lanes
# (r%16 >= 8) pull dens from r-8; all other lanes keep the memset 1.0.
_SHUF_MASK = [(_i - 8) if (_i % 16) >= 8 else 255 for _i in range(32)]


def _build_rbf3(ln_c0: float):
    nc = bacc.Bacc("TRN2", target_bir_lowering=False, debug=False)

    dlr_d = nc.dram_tensor(
        "dlr", [BPC, 12, N_IN + N_OUT], BF16, kind="ExternalInput"
    ).ap()
    dgrid_d = nc.dram_tensor("dgrid", [12, 2 * GRID], BF16, kind="ExternalInput").ap()
    y2_d = nc.dram_tensor("y2", [BPC, P, NXCH, 2], FP16, kind="ExternalInput").ap()
    wbig_d = nc.dram_tensor("wbig", [64, 2048], FP16, kind="ExternalInput").ap()
    out_d = nc.dram_tensor("out", [BPC, N_OUT, OUT_CH], FP16, kind="ExternalOutput").ap()

    with tile.TileContext(nc) as tc:
        with (
            tc.tile_pool(name="singles", bufs=1) as singles,
            tc.tile_pool(name="phip", bufs=4) as phip,
            tc.tile_pool(name="fcq", bufs=2) as fcq,
            tc.tile_pool(name="featp", bufs=2) as featp,
            tc.tile_pool(name="osbp", bufs=2) as osbp,
            tc.tile_pool(name="dps", bufs=2, space="PSUM") as dps,
            tc.tile_pool(name="aggp", bufs=2, space="PSUM") as aggp,
            tc.tile_pool(name="fop", bufs=2, space="PSUM") as fop,
        ):
            lnc0_sb = singles.tile([P, 1], F32)
            nc.vector.memset(lnc0_sb, ln_c0)
            dummy = singles.tile([1, 1], F32)
            nc.vector.memset(dummy, 0.0)
            dummy2 = singles.tile([1, 1], F32)
            nc.scalar.activation(
                out=dummy2, in_=dummy, func=mybir.ActivationFunctionType.Exp
            )
            nc.vector.tensor_copy(dummy, dummy2)  # keep dummy2 read

            # input DMAs spread across engines, dlr (the gating one) first
            dlr_all = singles.tile([12, BPC * (N_IN + N_OUT)], BF16)
            NLR = N_IN + N_OUT
            for bb in range(BPC):
                eng = nc.sync if bb == 0 else nc.scalar
                eng.dma_start(
                    out=dlr_all[:, bb * NLR : (bb + 1) * NLR],
                    in_=dlr_d[bb],
                )
            dgrid_sb = singles.tile([12, 2 * GRID], BF16)
            nc.sync.dma_start(out=dgrid_sb, in_=dgrid_d, single_packet=True)
            wbig_sb = singles.tile([64, 2048], FP16)
            nc.gpsimd.dma_start(out=wbig_sb, in_=wbig_d)
            d1r_sb = dgrid_sb[:, 0:GRID]
            d2l_sb = dgrid_sb[:, GRID : 2 * GRID]
            y2_all = singles.tile([P, BPC, NXCH, 2], FP16)
            nc.gpsimd.dma_start(
                out=y2_all,
                in_=bass.AP(
                    tensor=y2_d.tensor,
                    offset=y2_d.offset,
                    ap=[y2_d.ap[1], y2_d.ap[0], y2_d.ap[2], y2_d.ap[3]],
                ),
            )

            af16 = []
            for bb in range(BPC):
                a = singles.tile([P, 2], FP16, name=f"af16_{bb}")
                af16.append(a)

            def prologue(bb):
                base = bb * (N_IN + N_OUT)
                d1l_sb = dlr_all[:, base : base + N_IN]
                d1ps = fop.tile([P, 1024], F32, tag="fo", name="d1ps")
                for k in range(NXCH):
                    nc.tensor.matmul(
                        d1ps[:, 32 * k : 32 * k + GRID],
                        d1l_sb[:, k * P : (k + 1) * P],
                        d1r_sb,
                        start=True,
                        stop=True,
                    )
                phx = phip.tile([P, P], FP16, tag="phx")
                nc.scalar.activation(
                    out=phx,
                    in_=d1ps[:, 0:P],
                    func=mybir.ActivationFunctionType.Exp,
                    scale=-1.0,
                    bias=lnc0_sb,
                )
                # A accumulated 4x replicated along partitions: 16 tiny MMs
                aps = aggp.tile([P, 512], F32, tag="agg", name="aps")
                for r in range(4):
                    for k in range(NXCH):
                        nc.tensor.matmul(
                            aps[32 * r : 32 * r + GRID, 0:2],
                            phx[:, 32 * k : 32 * k + GRID],
                            y2_all[:, bb, k, :],
                            start=(k == 0),
                            stop=(k == NXCH - 1),
                            tile_position=(0, 32 * r),
                        )
                nc.scalar.copy(af16[bb], aps[:, 0:2])

            def block(bb, blk, fcomp):
                base = bb * (N_IN + N_OUT) + N_IN + blk * BLK
                d2ps = dps.tile([P, 512], F32, tag="d2")
                for s in range(4):
                    nc.tensor.matmul(
                        d2ps[32 * s : 32 * s + GRID, :],
                        d2l_sb,
                        dlr_all[:, base + 512 * s : base + 512 * (s + 1)],
                        start=True,
                        stop=True,
                        tile_position=(0, 32 * s),
                    )
                phi = phip.tile([P, 512], FP16, tag="phi")
                nc.scalar.activation(
                    out=phi,
                    in_=d2ps,
                    func=mybir.ActivationFunctionType.Exp,
                    scale=-1.0,
                )
                aggps = aggp.tile([P, 512], F32, tag="agg")
                for s in range(4):
                    nc.tensor.matmul(
                        aggps[32 * s : 32 * s + 2, :],
                        af16[bb][32 * s : 32 * s + GRID, :],
                        phi[32 * s : 32 * s + GRID, :],
                        start=True,
                        stop=True,
                        tile_position=(32 * s, 32 * s),
                    )
                dst = fcomp[:, 512 * blk : 512 * (blk + 1)]
                if blk % 2 == 0:
                    nc.scalar.copy(dst, aggps)
                else:
                    nc.vector.tensor_copy(dst, aggps)
            def batch_tail(bb, fcomp):
                # compaction: per s, one DMA moving both blocks+channels:
                # feats row 16s+8ch+4B+j <- fcomp[32s+ch, 512B+128j+p]
                feats = feats_of[bb]
                pf = fcomp.ap[0][0]
                pt = feats.ap[0][0]
                for s in range(4):
                    src = bass.AP(
                        tensor=fcomp.tensor,
                        offset=fcomp.offset + 32 * s * pf,
                        ap=[[pf, 2], [512, 2], [128, 4], [1, P]],
                    )
                    dstf = bass.AP(
                        tensor=feats.tensor,
                        offset=feats.offset + 16 * s * pt,
                        ap=[[pt, 16], [1, P]],
                    )
                    eng = nc.sync if s % 2 == 0 else nc.scalar
                    eng.dma_start(out=dstf, in_=src)
                # normalize: ddup = 1.0 except conv lanes get dens (row-8);
                # rall = 1/ddup; feats * rall -> fp16 (dens*1, conv/dens)
                ddup = featp.tile([64, P], F32, tag="dd")
                nc.gpsimd.memset(ddup, 1.0)
                nc.vector.stream_shuffle(ddup, feats, _SHUF_MASK)
                rall = featp.tile([64, P], F32, tag="ra")
                nc.vector.reciprocal_approx_fast(out=rall, in_=ddup)
                featsf = featp.tile([64, P], FP16, tag="ff")
                nc.vector.tensor_mul(featsf, feats, rall)
                # finale: 4 MMs vs block-diagonal W, 2x [128,1024] psum
                osb = osbp.tile([P, 2048], FP16, tag="osb")
                for h in range(2):
                    fout = fop.tile([P, 1024], F32, tag="fo")
                    for u in range(2):
                        for g in range(2):
                            nc.tensor.matmul(
                                fout[64 * g : 64 * (g + 1), 512 * u : 512 * (u + 1)],
                                featsf[:, 64 * g : 64 * (g + 1)],
                                wbig_sb[
                                    :, 1024 * h + 512 * u : 1024 * h + 512 * (u + 1)
                                ],
                                start=True,
                                stop=True,
                                tile_position=(0, 64 * g),
                            )
                    for u in range(2):
                        dsl = osb[:, 1024 * h + 512 * u : 1024 * h + 512 * (u + 1)]
                        if (h + u) % 2 == 0:
                            nc.scalar.copy(dsl, fout[:, 512 * u : 512 * (u + 1)])
                        else:
                            nc.vector.tensor_copy(
                                dsl, fout[:, 512 * u : 512 * (u + 1)]
                            )
                    # output DMA per half (one 2048-m block each)
                    sub = out_d[bb]
                    dsto = bass.AP(
                        tensor=sub.tensor,
                        offset=sub.offset + h * BLK * OUT_CH,
                        ap=[[16 * OUT_CH, P], [1, 1024]],
                    )
                    eng = nc.sync if h == 0 else nc.gpsimd
                    eng.dma_start(out=dsto, in_=osb[:, 1024 * h : 1024 * (h + 1)])

            feats_of = [featp.tile([64, P], F32, tag="ft", name=f"feats{i}") for i in range(BPC)]
            for bb in range(BPC):
                prologue(bb)
            fcomps = [
                fcq.tile([P, 1024], F32, tag="fq", name=f"fcomp{i}")
                for i in range(BPC)
            ]
            # interleave batches so both tails start as early as possible
            for blk in range(NBLK):
                for bb in range(BPC):
                    block(bb, blk, fcomps[bb])
            for bb in range(BPC):
                batch_tail(bb, fcomps[bb])

    nc.compile()
    return nc


def _split3(v):
    vh = v.astype(BF)
    r1 = v - vh.astype(np.float64)
    vm = r1.astype(BF)
    r2 = r1 - vm.astype(np.float64)
    vl = r2.astype(BF)
    return vh, vm, vl


def _d_rows(a, pts_t, pts_x):
    t = np.asarray(pts_t, dtype=np.float64)
    x = np.asarray(pts_x, dtype=np.float64)
    t2h, t2m, t2l = _split3(a * t * t)
    x2h, x2m, x2l = _split3(a * x * x)
    th, tm, tl = _split3(t)
    uh, um, ul = _split3(-2.0 * a * x)
    ones_t = np.ones_like(t, dtype=BF)
    ones_x = np.ones_like(x, dtype=BF)
    lhs = np.stack(
        [t2h, t2m, t2l, ones_t, ones_t, ones_t, th, th, tm, th, tm, tl], axis=-2
    )
    rhs = np.stack(
        [ones_x, ones_x, ones_x, x2h, x2m, x2l, uh, um, uh, ul, um, uh], axis=-2
    )
    return np.ascontiguousarray(lhs), np.ascontiguousarray(rhs)


def _prep_rbf3(x, y, t, a0, W):
    beta = 2.0 * a0
    s = 1.0 / (2.0 * np.sqrt(a0))
    margin = s * 5.68
    g = np.linspace(-margin, 1.0 + margin, GRID)
    h = g[1] - g[0]
    c0 = h * np.sqrt(4.0 * a0 / np.pi)
    ln_c0 = float(np.log(c0))

    # col (B*4+s)*512 + 128*j + p  <-  m = 2048*B + 16*p + 4*j + s
    tp = t.reshape(B, NBLK, P, 4, 4).transpose(0, 1, 4, 3, 2).reshape(B, N_OUT)
    d1_lhs, d1_rhs = _d_rows(beta, x, g)
    d2_lhs, d2_rhs = _d_rows(beta, g, tp)
    dlr = np.ascontiguousarray(np.concatenate([d1_lhs, d2_rhs], axis=-1))
    dgrid = np.ascontiguousarray(np.concatenate([d1_rhs, d2_lhs], axis=-1))
    y2 = np.empty((B, P, NXCH, 2), np.float16)
    y2[..., 0] = 1.0
    y2[..., 1] = y.reshape(B, NXCH, P).transpose(0, 2, 1)
    # wbig [64, 2048]: col block 64*(16B+4j+s): dens row 16s+4B+j = W0,
    # norm row 16s+8+4B+j = W1
    wbig = np.zeros((64, 2048), np.float16)
    w0 = W[:, 0].astype(np.float16)
    w1 = W[:, 1].astype(np.float16)
    for Bq in range(2):
        for j in range(4):
            for sq in range(4):
                cb = 64 * (16 * Bq + 4 * j + sq)
                wbig[16 * sq + 4 * Bq + j, cb : cb + 64] = w0
                wbig[16 * sq + 8 + 4 * Bq + j, cb : cb + 64] = w1

    in_maps = []
    for c in range(N_CORES):
        sl = slice(c * BPC, (c + 1) * BPC)
        in_maps.append(
            {
                "dlr": dlr[sl],
                "dgrid": dgrid,
                "y2": np.ascontiguousarray(y2[sl]),
                "wbig": wbig,
            }
        )
    return in_maps, ln_c0


def kernel(x, y, t, sigma, W, b, _trace=False):
    x = np.ascontiguousarray(x[..., 0], dtype=np.float32)
    y = np.ascontiguousarray(y[..., 0], dtype=np.float32)
    t = np.ascontiguousarray(t[..., 0], dtype=np.float32)
    scales = np.exp(sigma.astype(np.float32))
    a0 = float(np.float32(0.5) / (scales[0] * scales[0]))
    a1 = float(np.float32(0.5) / (scales[1] * scales[1]))
    assert a0 == a1, "v3 kernel requires shared length scale"

    in_maps, ln_c0 = _prep_rbf3(x, y, t, a0, W)
    key = ("rbf3", ln_c0)
    if key not in _CACHE:
        _CACHE[key] = _build_rbf3(ln_c0)
    nc = _CACHE[key]
    res = run_bass_kernel_spmd(
        nc, in_maps, core_ids=list(range(N_CORES)), trace=_trace
    )
    out = np.concatenate([r["out"] for r in res.results], axis=0)
    kernel.last_exec_time_ns = res.exec_time_ns
    kernel.last_results = res
    out = out.reshape(B, N_OUT, OUT_CH).astype(np.float32)
    out += b.astype(np.float32)[None, None, :]
    return np.ascontiguousarray(out)
